# revision 48
# baseline (speedup 1.0000x reference)
"""Trainium2 Bass kernel for nn_Expand_36610301231376.

kernel(**inputs) takes the FULL unsharded inputs (as in reference.setup_inputs)
and returns the FULL (16, 512, 56, 56) float32 output.

Strategy: pure data parallel over batch B=16 across 8 NeuronCores (2 batches
per core). Key algebraic restructurings vs a direct implementation:

- The q-projection is composed through the rank-149 conv bottleneck:
  Bq = x @ (diag(g1) Wq^T)  [149, 512] per batch (cheap), then
  q_raw^T = Bq^T routed through W1^T per token chunk (2 matmuls per 128-wide
  output chunk instead of 6), eliminating the xe = conv1(x) materialization.
- LN1 statistics come from the tiny Gram matrix G = x x^T [149,149] and the
  row-sum s_x = sum_d x: sum_d xe = W1 @ s_x, sum_d xe^2 = diag(W1 G W1^T),
  computed per chunk with 8 small matmuls instead of 12 full reductions plus
  squares.
- k and v are computed directly from raw (bf16) y so the bulk of PE work has
  no dependency on the LN statistics chain: k = (Wkg2 @ y + uk (x) (-mu2)) *
  rsqrt_bcast + ck, and v applies the LN affine in token-major space (after
  the PE transposes) where the per-token stats are per-partition scalars.
- LN mean/rsqrt chains for both layernorms run in one [1,2,448] free-dim-
  stacked pass; rsqrt is exp(-0.5 ln(v+eps)) and softmax division is
  exp(z - ln(den)) so every activation lives in one table set (no reloads)
  and no slow DVE reciprocal is needed.
- The conv bias b_conv1 cancels in LN; LN gammas/betas, positional encodings,
  projection biases and the 1/sqrt(512) attention scale are folded into
  host-precomputed constants (cq, ck, pe2t).
- The attention mask is accumulated into PSUM via an identity matmul; emission
  is software-pipelined 3 units deep (attention of unit i overlaps q/k/v of
  i+1, stats of i+2, DMA of i+3).
- The residual +y is applied on the host; the device returns bf16 attention
  output, halving output DMA.
"""
import sys

if "/opt/trn_rl_repo" not in sys.path:
    sys.path.insert(0, "/opt/trn_rl_repo")

import numpy as np
import orjson

# ----------------------------------------------------------------------------
# BIR post-pass: this container's walrus build supports only ONE sync-wait per
# instruction; split multi-wait instructions into single-wait NoOps.
# ----------------------------------------------------------------------------
_wcounter = [0]


def _split_block(instructions):
    out, changed = [], False
    for inst in instructions:
        si = inst.get("sync_info")
        waits = (si or {}).get("on_wait") or []
        if len(waits) > 1:
            changed = True
            for w in waits[:-1]:
                _wcounter[0] += 1
                nop = {
                    "engine": inst["engine"], "ins": [], "outs": [],
                    "name": f"I-wsplit-{_wcounter[0]}", "opcode": "NoOp",
                    "sync_info": {"on_update": [], "on_wait": [w]},
                }
                if "debug" in inst:
                    nop["debug"] = inst["debug"]
                out.append(nop)
            si["on_wait"] = [waits[-1]]
        out.append(inst)
    return out, changed


def _split_multi_waits_json(bir_json: bytes) -> bytes:
    m = orjson.loads(bir_json)
    changed = False
    for fn in m.get("functions", []):
        for blk in fn.get("blocks", []):
            insts = blk.get("instructions")
            if insts:
                blk["instructions"], ch = _split_block(insts)
                changed = changed or ch
    return orjson.dumps(m) if changed else bir_json


def _install_patch():
    import concourse.bass as bass

    if getattr(bass.Bass, "_wait_split_installed", False):
        return
    orig = bass.Bass.to_json_bytes

    def to_json_bytes(self):
        return _split_multi_waits_json(orig(self))

    bass.Bass.to_json_bytes = to_json_bytes
    bass.Bass._wait_split_installed = True


# ----------------------------------------------------------------------------
# Problem constants (hardcoded from the problem spec)
# ----------------------------------------------------------------------------
B = 16
N_CORES = 8
B_LOC = B // N_CORES
T_LEN, T_DIM = 149, 768
H = W = 56
S_DIM = 512
N_TOK = H * W           # 3136
CH = 448                # tokens per chunk (8 image rows)
NCHUNK = N_TOK // CH    # 7
NBLK = CH // 112        # 4 two-row attention blocks per chunk
EPS = 1e-5


# ----------------------------------------------------------------------------
# Device program
# ----------------------------------------------------------------------------
def _build_program(apply_g2: bool):
    import concourse.bass as bass
    import concourse.tile as tile
    from concourse import mybir

    F32 = mybir.dt.float32
    BF16 = mybir.dt.bfloat16
    AF = mybir.ActivationFunctionType
    OP = mybir.AluOpType

    nc = bass.Bass(trn_type="TRN2", target_bir_lowering=False, debug=False)
    din = {}
    for name, shape, dt_ in [
        ("x0", (128, B_LOC, T_DIM), BF16), ("x1", (32, B_LOC, T_DIM), BF16),
        ("xdl", (128, 6, B_LOC, T_LEN), BF16),
        ("wqg", (128, 6, S_DIM), BF16),
        ("w1t", (128, 2, N_TOK), BF16),
        ("uq", (1, S_DIM), BF16), ("uk", (1, S_DIM), BF16),
        ("wkt", (128, 4, S_DIM), BF16),
        ("ones", (128, 128), BF16), ("ident", (128, 128), BF16),
        ("cq", (128, 4, N_TOK), BF16), ("ck", (128, 4, N_TOK), BF16),
        ("pe2t", (112, NCHUNK, 4, S_DIM), BF16),
        ("g2bt", (112, S_DIM), BF16),
        ("masks", (112, 112), BF16),
        ("icol", (128, 2), BF16),
        ("ybf", (B_LOC, 128, 4, N_TOK), BF16),
    ]:
        din[name] = nc.dram_tensor(name, list(shape), dt_, kind="ExternalInput").ap()
    dout = nc.dram_tensor("out", [B_LOC, 128, 4, N_TOK], BF16,
                          kind="ExternalOutput").ap()

    from contextlib import ExitStack

    with nc.allow_low_precision(reason="bf16 matmul operands, fp32 accumulate"), \
         tile.TileContext(nc) as tc, ExitStack() as ctx:
        singles = ctx.enter_context(tc.tile_pool(name="singles", bufs=1))
        io3 = ctx.enter_context(tc.tile_pool(name="io3", bufs=3))
        io2 = ctx.enter_context(tc.tile_pool(name="io2", bufs=2))
        wk2 = ctx.enter_context(tc.tile_pool(name="wk2", bufs=2))
        att = ctx.enter_context(tc.tile_pool(name="att", bufs=3))
        sc = ctx.enter_context(tc.tile_pool(name="sc", bufs=2))
        ps_mm = ctx.enter_context(tc.tile_pool(name="ps_mm", bufs=3, space="PSUM"))
        ps_st = ctx.enter_context(tc.tile_pool(name="ps_st", bufs=2, space="PSUM"))
        ps_att = ctx.enter_context(tc.tile_pool(name="ps_att", bufs=2, space="PSUM"))

        def load(name, shape, dt_):
            t = singles.tile(list(shape), dt_, tag=name)
            nc.sync.dma_start(out=t, in_=din[name])
            return t

        x0 = load("x0", (128, B_LOC, T_DIM), BF16)
        x1 = load("x1", (32, B_LOC, T_DIM), BF16)
        xdl = load("xdl", (128, 6, B_LOC, T_LEN), BF16)
        wqg = load("wqg", (128, 6, S_DIM), BF16)
        w1t = load("w1t", (128, 2, N_TOK), BF16)
        uq = load("uq", (1, S_DIM), BF16)
        uk = load("uk", (1, S_DIM), BF16)
        wkt = load("wkt", (128, 4, S_DIM), BF16)
        ones = load("ones", (128, 128), BF16)
        ident = load("ident", (128, 128), BF16)
        g2bt = load("g2bt", (112, S_DIM), BF16)
        masks = load("masks", (112, 112), BF16)
        ones_col = ones[:, 0:1]
        ones_row = ones[0:1, :]
        eps1 = singles.tile([1, 1], F32)
        nc.vector.memset(eps1, EPS)
        onef = singles.tile([1, 1], F32)
        nc.vector.memset(onef, 1.0)
        eps112 = singles.tile([112, 1], F32)
        nc.vector.memset(eps112, EPS)
        icol = load("icol", (128, 2), BF16)

        # ---- per-batch preamble: s_x, Bq = x @ Wqg, G = x x^T ----
        pre = {}
        for b in range(B_LOC):
            sx0 = singles.tile([128, 1], F32, tag=f"sx0_{b}")
            nc.vector.tensor_reduce(out=sx0, in_=x0[:, b, :],
                                    axis=mybir.AxisListType.X, op=OP.add)
            sx1 = singles.tile([32, 1], F32, tag=f"sx1_{b}")
            nc.vector.tensor_reduce(out=sx1, in_=x1[:, b, :],
                                    axis=mybir.AxisListType.X, op=OP.add)
            sxb0 = singles.tile([128, 1], BF16, tag=f"sxb0_{b}")
            nc.scalar.activation(out=sxb0, in_=sx0, func=AF.Copy,
                                 scale=1.0 / T_DIM)
            sxb1 = singles.tile([32, 1], BF16, tag=f"sxb1_{b}")
            nc.scalar.activation(out=sxb1, in_=sx1, func=AF.Copy,
                                 scale=1.0 / T_DIM)

            bq_ps0 = ps_mm.tile([128, S_DIM], F32, tag="mm")
            for dc in range(6):
                nc.tensor.matmul(bq_ps0, xdl[:, dc, b, 0:128], wqg[:, dc, :],
                                 start=(dc == 0), stop=(dc == 5))
            bq0 = singles.tile([128, S_DIM], BF16, tag=f"bq0_{b}")
            nc.scalar.activation(out=bq0, in_=bq_ps0, func=AF.Copy)
            bq_ps1 = ps_mm.tile([21, S_DIM], F32, tag="mm")
            for dc in range(6):
                nc.tensor.matmul(bq_ps1, xdl[:, dc, b, 128:149], wqg[:, dc, :],
                                 start=(dc == 0), stop=(dc == 5))
            bq1 = singles.tile([21, S_DIM], BF16, tag=f"bq1_{b}")
            nc.scalar.activation(out=bq1, in_=bq_ps1, func=AF.Copy)

            g_ps0 = ps_mm.tile([128, T_LEN], F32, tag="mm")
            for dc in range(6):
                nc.tensor.matmul(g_ps0, xdl[:, dc, b, 0:128], xdl[:, dc, b, :],
                                 start=(dc == 0), stop=(dc == 5))
            g0 = singles.tile([128, T_LEN], BF16, tag=f"g0_{b}")
            nc.scalar.activation(out=g0, in_=g_ps0, func=AF.Copy)
            g_ps1 = ps_mm.tile([21, T_LEN], F32, tag="mm")
            for dc in range(6):
                nc.tensor.matmul(g_ps1, xdl[:, dc, b, 128:149], xdl[:, dc, b, :],
                                 start=(dc == 0), stop=(dc == 5))
            g1t = singles.tile([21, T_LEN], BF16, tag=f"g1_{b}")
            nc.scalar.activation(out=g1t, in_=g_ps1, func=AF.Copy)
            pre[b] = (sxb0, sxb1, bq0, bq1, g0, g1t)

        # ---- per chunk x batch phases, software-pipelined over units ----
        def em_ld(u):
            if u["first"]:
                cq_t = io2.tile([128, 4, CH], BF16, tag="cq", bufs=3)
                nc.sync.dma_start(out=cq_t, in_=din["cq"][:, :, u["cols"]])
                ck_t = io2.tile([128, 4, CH], BF16, tag="ck", bufs=3)
                nc.sync.dma_start(out=ck_t, in_=din["ck"][:, :, u["cols"]])
                pe2_t = io2.tile([112, 4, S_DIM], BF16, tag="pe2", bufs=3)
                nc.sync.dma_start(out=pe2_t, in_=din["pe2t"][:, u["ich"], :, :])
                chunk_io[u["ich"]] = (cq_t, ck_t, pe2_t)
            u["cq_t"], u["ck_t"], u["pe2_t"] = chunk_io[u["ich"]]
            ybf = io3.tile([128, 4, CH], BF16, tag="ybf")
            nc.sync.dma_start(out=ybf, in_=din["ybf"][u["b"], :, :, u["cols"]])
            u["ybf"] = ybf

        def em_stats(u):
            b, cols, ybf = u["b"], u["cols"], u["ybf"]
            sxb0, sxb1, bq0, bq1, g0, g1t = pre[b]
            # stat segments in SBUF free dim: 0 = LN1 mean, 1 = LN1 E[x^2]
            st_sb = sc.tile([1, 2, CH], F32, tag="stsb")
            s1p = ps_st.tile([1, CH], F32, tag="st")
            nc.tensor.matmul(s1p, sxb0, w1t[:, 0, cols],
                             start=True, stop=False)
            nc.tensor.matmul(s1p, sxb1[:21, :], w1t[:21, 1, cols],
                             start=False, stop=True)
            nc.scalar.activation(out=st_sb[:, 0, :], in_=s1p, func=AF.Copy)
            # LN1 sumsq via Gram: T = G @ W1T, then colsum(W1T * T)
            t0 = ps_mm.tile([128, CH], F32, tag="mm")
            nc.tensor.matmul(t0, g0[:, 0:128], w1t[:, 0, cols],
                             start=True, stop=False)
            nc.tensor.matmul(t0, g1t[:, 0:128], w1t[:21, 1, cols],
                             start=False, stop=True)
            v0 = wk2.tile([128, CH], BF16, tag="v0")
            nc.vector.tensor_tensor(out=v0, in0=t0, in1=w1t[:, 0, cols],
                                    op=OP.mult)
            t1 = ps_mm.tile([21, CH], F32, tag="mm")
            nc.tensor.matmul(t1, g0[:, 128:149], w1t[:, 0, cols],
                             start=True, stop=False)
            nc.tensor.matmul(t1, g1t[:, 128:149], w1t[:21, 1, cols],
                             start=False, stop=True)
            v1 = wk2.tile([21, CH], BF16, tag="v1")
            nc.vector.tensor_tensor(out=v1, in0=t1, in1=w1t[:21, 1, cols],
                                    op=OP.mult)
            q1p = ps_st.tile([1, CH], F32, tag="st")
            nc.tensor.matmul(q1p, icol[:, 0:1], v0, start=True, stop=False)
            nc.tensor.matmul(q1p, icol[:21, 0:1], v1, start=False, stop=True)
            nc.scalar.activation(out=st_sb[:, 1, :], in_=q1p, func=AF.Copy)
            u["st_sb"] = st_sb

        def em_vt(u):
            ybf = u["ybf"]
            vts = wk2.tile([112, NBLK, S_DIM], BF16, tag="vts")
            m2c = sc.tile([112, 4], F32, tag="m2c")
            q2c = sc.tile([112, 4], F32, tag="q2c")
            for blk in range(NBLK):
                tb = slice(blk * 112, (blk + 1) * 112)
                pt = ps_att.tile([112, S_DIM], BF16, tag="at")
                for co in range(4):
                    nc.tensor.transpose(pt[:, co * 128:(co + 1) * 128],
                                        ybf[:, co, tb], ident)
                nc.scalar.activation(out=vts[:, blk, :], in_=pt, func=AF.Copy,
                                     accum_out=m2c[:, blk:blk + 1])
                scr = att.tile([112, S_DIM], BF16, tag="scr")
                nc.scalar.activation(out=scr, in_=vts[:, blk, :],
                                     func=AF.Square,
                                     accum_out=q2c[:, blk:blk + 1])
            mcn = sc.tile([112, 4], F32, tag="mcn")
            nc.scalar.activation(out=mcn, in_=m2c, func=AF.Copy,
                                 scale=-1.0 / S_DIM)
            msqc = sc.tile([112, 4], F32, tag="msqc")
            nc.vector.tensor_mul(out=msqc, in0=mcn, in1=mcn)
            varc = sc.tile([112, 4], F32, tag="varc")
            nc.vector.tensor_scalar(out=varc, in0=q2c, scalar1=1.0 / S_DIM,
                                    scalar2=None, op0=OP.mult)
            nc.vector.tensor_tensor(out=varc, in0=varc, in1=msqc,
                                    op=OP.subtract)
            lnc = sc.tile([112, 4], F32, tag="lnc")
            nc.scalar.activation(out=lnc, in_=varc, func=AF.Ln, bias=eps112)
            rcol = sc.tile([112, 4], F32, tag="rcol")
            nc.scalar.activation(out=rcol, in_=lnc, func=AF.Exp, scale=-0.5)
            c2col = sc.tile([112, 4], F32, tag="c2col")
            nc.vector.tensor_mul(out=c2col, in0=mcn, in1=rcol)
            rcb = sc.tile([112, 4], BF16, tag="rcb")
            nc.scalar.activation(out=rcb, in_=rcol, func=AF.Copy)
            mcb = sc.tile([112, 4], BF16, tag="mcb")
            nc.scalar.activation(out=mcb, in_=mcn, func=AF.Copy)
            rowp = ps_st.tile([1, 2, CH], BF16, tag="st")
            for blk in range(NBLK):
                tb = slice(blk * 112, (blk + 1) * 112)
                nc.tensor.transpose(rowp[:, 0, tb], rcb[:, blk:blk + 1],
                                    ident[:112, :112])
                nc.tensor.transpose(rowp[:, 1, tb], mcb[:, blk:blk + 1],
                                    ident[:112, :112])
            rowsb = sc.tile([1, 2, CH], BF16, tag="rowsb")
            nc.scalar.activation(out=rowsb, in_=rowp, func=AF.Copy)
            u["vts"], u["rcol"], u["c2col"] = vts, rcol, c2col
            u["rowsb"] = rowsb

        def em_chain(u):
            stp = u["st_sb"]
            mneg = sc.tile([1, 1, CH], BF16, tag="mrow")
            nc.scalar.activation(out=mneg, in_=stp[:, 0:1, :], func=AF.Copy,
                                 scale=-1.0)
            msq = sc.tile([1, 1, CH], F32, tag="msq")
            nc.vector.tensor_mul(out=msq, in0=stp[:, 0:1, :],
                                 in1=stp[:, 0:1, :])
            vrow = sc.tile([1, 1, CH], F32, tag="vrow")
            nc.vector.tensor_tensor(out=vrow, in0=stp[:, 1:2, :], in1=msq,
                                    op=OP.subtract)
            lv = sc.tile([1, 1, CH], F32, tag="lv")
            nc.scalar.activation(out=lv, in_=vrow, func=AF.Ln, bias=eps1)
            rrow = sc.tile([1, 1, CH], BF16, tag="rr")
            nc.scalar.activation(out=rrow, in_=lv, func=AF.Exp, scale=-0.5)
            u["mneg"], u["rrow"] = mneg, rrow

        def em_bcast(u):
            rrow, rowsb = u["rrow"], u["rowsb"]
            rb = wk2.tile([128, 2, CH], BF16, tag="rb")
            r1b_ps = ps_st.tile([128, CH], F32, tag="bc", bufs=1)
            nc.tensor.matmul(r1b_ps, ones_row, rrow[:, 0, :],
                             start=True, stop=True)
            nc.scalar.activation(out=rb[:, 0, :], in_=r1b_ps, func=AF.Copy)
            r2b_ps = ps_st.tile([128, CH], F32, tag="bc", bufs=1)
            nc.tensor.matmul(r2b_ps, ones_row, rowsb[:, 0, :],
                             start=True, stop=True)
            nc.scalar.activation(out=rb[:, 1, :], in_=r2b_ps, func=AF.Copy)
            u["r1b"], u["r2b"] = rb[:, 0, :], rb[:, 1, :]

        def em_q(u):
            b, cols = u["b"], u["cols"]
            mneg, r1b, cq_t = u["mneg"], u["r1b"], u["cq_t"]
            _, _, bq0, bq1, _, _ = pre[b]
            q = wk2.tile([128, 4, CH], BF16, tag="q")
            for oc in range(4):
                ocs = slice(oc * 128, (oc + 1) * 128)
                pq = ps_mm.tile([128, CH], F32, tag="mm")
                nc.tensor.matmul(pq, bq0[:, ocs], w1t[:, 0, cols],
                                 start=True, stop=False)
                nc.tensor.matmul(pq, bq1[:, ocs], w1t[:21, 1, cols],
                                 start=False, stop=False)
                nc.tensor.matmul(pq, uq[:, ocs], mneg[:, 0, :],
                                 start=False, stop=True)
                tmp = att.tile([128, CH], BF16, tag="qt")
                nc.vector.tensor_mul(out=tmp, in0=pq, in1=r1b)
                nc.vector.tensor_add(out=q[:, oc, :], in0=tmp,
                                     in1=cq_t[:, oc, :])
            u["q"] = q

        def em_k(u):
            ybf, r2b, ck_t = u["ybf"], u["r2b"], u["ck_t"]
            m2row = u["rowsb"][:, 1, :]
            k = wk2.tile([128, 4, CH], BF16, tag="k")
            for oc in range(4):
                ocs = slice(oc * 128, (oc + 1) * 128)
                pk = ps_mm.tile([128, CH], F32, tag="mm")
                for kc in range(4):
                    nc.tensor.matmul(pk, wkt[:, kc, ocs], ybf[:, kc, :],
                                     start=(kc == 0), stop=False)
                nc.tensor.matmul(pk, uk[:, ocs], m2row,
                                 start=False, stop=True)
                kt = att.tile([128, CH], BF16, tag="kt")
                nc.vector.tensor_mul(out=kt, in0=pk, in1=r2b)
                nc.vector.tensor_add(out=k[:, oc, :], in0=kt,
                                     in1=ck_t[:, oc, :])
            u["k"] = k

        def em_v(u):
            vts, rcol, c2col = u["vts"], u["rcol"], u["c2col"]
            pe2_t = u["pe2_t"]
            v = wk2.tile([112, NBLK, S_DIM], BF16, tag="v")
            for blk in range(NBLK):
                v1t = att.tile([112, S_DIM], BF16, tag="v1t")
                nc.vector.tensor_scalar(out=v1t, in0=vts[:, blk, :],
                                        scalar1=rcol[:, blk:blk + 1],
                                        scalar2=c2col[:, blk:blk + 1],
                                        op0=OP.mult, op1=OP.add)
                if apply_g2:
                    v1g = att.tile([112, S_DIM], BF16, tag="v1g")
                    nc.vector.tensor_mul(out=v1g, in0=v1t, in1=g2bt)
                    v1t = v1g
                nc.gpsimd.tensor_add(out=v[:, blk, :], in0=v1t,
                                     in1=pe2_t[:, blk, :])
            u["v"] = v

        def em_att_a(u):
            q, k = u["q"], u["k"]
            u["attn"] = []
            for blk in range(NBLK):
                tb = slice(blk * 112, (blk + 1) * 112)
                psc = ps_att.tile([112, 112], F32, tag="at")
                nc.tensor.matmul(psc, ident[:112, :112], masks,
                                 start=True, stop=False)
                for oc in range(4):
                    nc.tensor.matmul(psc, q[:, oc, tb], k[:, oc, tb],
                                     start=False, stop=(oc == 3))
                den = sc.tile([112, 1], F32, tag="den")
                e_b = att.tile([112, 112], BF16, tag="eb")
                nc.scalar.activation(out=e_b, in_=psc, func=AF.Exp,
                                     accum_out=den)
                ld = sc.tile([112, 1], F32, tag="ld")
                nc.scalar.activation(out=ld, in_=den, func=AF.Ln)
                nld = sc.tile([112, 1], F32, tag="nld")
                nc.scalar.activation(out=nld, in_=ld, func=AF.Copy,
                                     scale=-1.0)
                attn = att.tile([112, 112], BF16, tag="attn")
                nc.scalar.activation(out=attn, in_=psc, func=AF.Exp,
                                     bias=nld)
                u["attn"].append(attn)

        def em_att_b(u):
            b, cols, v = u["b"], u["cols"], u["v"]
            out_t = io2.tile([128, 4, CH], BF16, tag="out")
            for blk in range(NBLK):
                tb = slice(blk * 112, (blk + 1) * 112)
                pat = ps_att.tile([112, 112], BF16, tag="at")
                nc.tensor.transpose(pat, u["attn"][blk], ident[:112, :112])
                attnT = att.tile([112, 112], BF16, tag="attnT")
                nc.vector.tensor_copy(out=attnT, in_=pat)
                pav = ps_att.tile([128, 4, 112], F32, tag="at")
                for co in range(4):
                    nc.tensor.matmul(pav[:, co, :],
                                     v[:, blk, co * 128:(co + 1) * 128],
                                     attnT, start=True, stop=True)
                nc.vector.tensor_copy(out=out_t[:, :, tb], in_=pav)
            nc.sync.dma_start(out=dout[b, :, :, cols], in_=out_t)

        units = []
        for ich in range(NCHUNK):
            cols = slice(ich * CH, (ich + 1) * CH)
            for b in range(B_LOC):
                units.append({"b": b, "ich": ich, "cols": cols,
                              "first": b == 0})
        chunk_io = {}
        n = len(units)
        for i in range(-3, n):
            if 0 <= i < n:
                em_att_a(units[i])
                em_att_b(units[i])
            if 0 <= i + 1 < n:
                em_q(units[i + 1])
                em_k(units[i + 1])
                em_v(units[i + 1])
            if 0 <= i + 2 < n:
                em_stats(units[i + 2])
                em_vt(units[i + 2])
                em_chain(units[i + 2])
                em_bcast(units[i + 2])
            if 0 <= i + 3 < n:
                em_ld(units[i + 3])
    return nc


# ----------------------------------------------------------------------------
# Host-side preparation
# ----------------------------------------------------------------------------
def _make_const_inputs(W_conv1, b_conv1, ln1_g, ln1_b, ln2_g, ln2_b,
                       pe_wave, pe_spec, Wq, bq, Wk, bk):
    import ml_dtypes
    f = np.float32
    bf = ml_dtypes.bfloat16
    s = np.float32(S_DIM) ** np.float32(-0.25)

    w1t = np.zeros((128, 2, N_TOK), dtype=f)
    w1T = W_conv1.T.astype(f)
    w1t[:, 0, :] = w1T[:128]
    w1t[:21, 1, :] = w1T[128:]

    # Wqg[d, c] = Wq[c, d] * g1[d] * s, laid out [128, 6, 512]
    wqg = (Wq.T * ln1_g[:, None]).astype(f) * s
    wqg = wqg.reshape(6, 128, S_DIM).transpose(1, 0, 2).copy()
    uq = (Wq @ ln1_g).astype(f)[None, :] * s

    pe_w = pe_wave.reshape(T_DIM, N_TOK).astype(f)
    cq = (Wq @ (ln1_b[:, None] + pe_w)).astype(f) * s + (bq[:, None] * s).astype(f)
    cq = cq.reshape(4, 128, N_TOK).transpose(1, 0, 2).copy()

    # wkt rows scaled by g2 (k-side gamma fold)
    wkt = (Wk.T * (s * ln2_g[:, None])).astype(f)
    wkt = wkt.reshape(4, 128, S_DIM).transpose(1, 0, 2).copy()
    uk = (Wk @ ln2_g).astype(f)[None, :] * s
    apply_g2 = not np.allclose(ln2_g, 1.0)

    pe2_full = (pe_spec.reshape(S_DIM, N_TOK) + ln2_b[:, None]).astype(f)
    ck = ((Wk * ln2_g[None, :]) @ pe2_full) * s + (bk * s)[:, None]
    ck = ck.astype(f).reshape(4, 128, N_TOK).transpose(1, 0, 2).copy()
    pe2t = pe2_full.reshape(S_DIM, NCHUNK, NBLK, 112)
    pe2t = pe2t.transpose(3, 1, 2, 0).copy()
    g2bt = np.broadcast_to(ln2_g[None, :].astype(f), (112, S_DIM)).copy()

    masks = np.full((112, 112), -1e30, dtype=f)
    for sb in range(2):
        masks[sb * 56:(sb + 1) * 56, sb * 56:(sb + 1) * 56] = 0.0

    return {
        "w1t": w1t.astype(bf), "wqg": wqg.astype(bf), "uq": uq.astype(bf),
        "uk": uk.astype(bf), "cq": cq.astype(bf), "wkt": wkt.astype(bf),
        "ck": ck.astype(bf), "pe2t": pe2t.astype(bf),
        "g2bt": g2bt.astype(bf), "masks": masks.astype(bf),
        "ones": np.ones((128, 128), dtype=bf),
        "ident": np.eye(128, dtype=bf),
        "icol": np.stack([np.full(128, 1.0 / T_DIM, dtype=f),
                          np.full(128, 1.0 / S_DIM, dtype=f)],
                         axis=1).astype(bf),
        "_apply_g2": apply_g2,
    }


def _make_core_inputs(consts, x_shard, y_shard):
    import ml_dtypes
    bf = ml_dtypes.bfloat16
    x0 = x_shard[:, :128, :].transpose(1, 0, 2).astype(bf).copy()
    x1 = np.zeros((32, B_LOC, T_DIM), dtype=bf)
    x1[:21] = x_shard[:, 128:, :].transpose(1, 0, 2).astype(bf)
    # x_dl[p, dc, b, l] = x[b, l, dc*128+p]
    xdl = x_shard.transpose(2, 0, 1).reshape(6, 128, B_LOC, T_LEN)
    xdl = xdl.transpose(1, 0, 2, 3).astype(bf).copy()
    ybf = y_shard.reshape(B_LOC, 4, 128, N_TOK).transpose(0, 2, 1, 3)
    ybf = ybf.astype(bf).copy()
    m = {"x0": x0, "x1": x1, "xdl": xdl, "ybf": ybf}
    m.update({k: v for k, v in consts.items() if not k.startswith("_")})
    return m


_cached_nc = [None]


def kernel(x, y, W_conv1, b_conv1, ln1_g, ln1_b, ln2_g, ln2_b,
           pe_wave, pe_spec, Wq, bq, Wk, bk):
    _install_patch()
    from concourse.bass_utils import run_bass_kernel_spmd

    x = np.asarray(x, dtype=np.float32)
    y = np.asarray(y, dtype=np.float32)
    consts = _make_const_inputs(
        np.asarray(W_conv1, np.float32), np.asarray(b_conv1, np.float32),
        np.asarray(ln1_g, np.float32), np.asarray(ln1_b, np.float32),
        np.asarray(ln2_g, np.float32), np.asarray(ln2_b, np.float32),
        np.asarray(pe_wave, np.float32), np.asarray(pe_spec, np.float32),
        np.asarray(Wq, np.float32), np.asarray(bq, np.float32),
        np.asarray(Wk, np.float32), np.asarray(bk, np.float32))
    in_maps = [
        _make_core_inputs(consts, x[B_LOC * i:B_LOC * (i + 1)],
                          y[B_LOC * i:B_LOC * (i + 1)])
        for i in range(N_CORES)
    ]

    if _cached_nc[0] is None:
        _cached_nc[0] = _build_program(consts["_apply_g2"])
    nc = _cached_nc[0]

    res = run_bass_kernel_spmd(nc, in_maps, core_ids=list(range(N_CORES)))
    outs = []
    for i in range(N_CORES):
        o = np.asarray(res.results[i]["out"], dtype=np.float32)
        outs.append(o.transpose(0, 2, 1, 3).reshape(B_LOC, S_DIM, H, W))
    return (np.concatenate(outs, axis=0) + y).astype(np.float32)


# revision 49
# speedup vs baseline: 1.1826x; 1.1826x over previous
"""Trainium2 Bass kernel for nn_Expand_36610301231376.

kernel(**inputs) takes the FULL unsharded inputs (as in reference.setup_inputs)
and returns the FULL (16, 512, 56, 56) float32 output.

Strategy: pure data parallel over batch B=16 across 8 NeuronCores (2 batches
per core). Key algebraic restructurings vs a direct implementation:

- The q-projection is composed through the rank-149 conv bottleneck:
  Bq = x @ (diag(g1) Wq^T)  [149, 512] per batch (cheap), then
  q_raw^T = Bq^T routed through W1^T per token chunk (2 matmuls per 128-wide
  output chunk instead of 6), eliminating the xe = conv1(x) materialization.
- LN1 statistics come from the tiny Gram matrix G = x x^T [149,149] and the
  row-sum s_x = sum_d x: sum_d xe = W1 @ s_x, sum_d xe^2 = diag(W1 G W1^T),
  computed per chunk with 8 small matmuls instead of 12 full reductions plus
  squares.
- k and v are computed directly from raw (bf16) y so the bulk of PE work has
  no dependency on the LN statistics chain: k = (Wkg2 @ y + uk (x) (-mu2)) *
  rsqrt_bcast + ck, and v applies the LN affine in token-major space (after
  the PE transposes) where the per-token stats are per-partition scalars.
- LN mean/rsqrt chains for both layernorms run in one [1,2,448] free-dim-
  stacked pass; rsqrt is exp(-0.5 ln(v+eps)) and softmax division is
  exp(z - ln(den)) so every activation lives in one table set (no reloads)
  and no slow DVE reciprocal is needed.
- The conv bias b_conv1 cancels in LN; LN gammas/betas, positional encodings,
  projection biases and the 1/sqrt(512) attention scale are folded into
  host-precomputed constants (cq, ck, pe2t).
- The attention mask is accumulated into PSUM via an identity matmul; emission
  is software-pipelined 3 units deep (attention of unit i overlaps q/k/v of
  i+1, stats of i+2, DMA of i+3).
- The residual +y is applied on the host; the device returns bf16 attention
  output, halving output DMA.
"""
import sys

if "/opt/trn_rl_repo" not in sys.path:
    sys.path.insert(0, "/opt/trn_rl_repo")

import numpy as np
import orjson

# ----------------------------------------------------------------------------
# BIR post-pass: this container's walrus build supports only ONE sync-wait per
# instruction; split multi-wait instructions into single-wait NoOps.
# ----------------------------------------------------------------------------
_wcounter = [0]


def _split_block(instructions):
    out, changed = [], False
    for inst in instructions:
        si = inst.get("sync_info")
        waits = (si or {}).get("on_wait") or []
        if len(waits) > 1:
            changed = True
            for w in waits[:-1]:
                _wcounter[0] += 1
                nop = {
                    "engine": inst["engine"], "ins": [], "outs": [],
                    "name": f"I-wsplit-{_wcounter[0]}", "opcode": "NoOp",
                    "sync_info": {"on_update": [], "on_wait": [w]},
                }
                if "debug" in inst:
                    nop["debug"] = inst["debug"]
                out.append(nop)
            si["on_wait"] = [waits[-1]]
        out.append(inst)
    return out, changed


def _split_multi_waits_json(bir_json: bytes) -> bytes:
    m = orjson.loads(bir_json)
    changed = False
    for fn in m.get("functions", []):
        for blk in fn.get("blocks", []):
            insts = blk.get("instructions")
            if insts:
                blk["instructions"], ch = _split_block(insts)
                changed = changed or ch
    return orjson.dumps(m) if changed else bir_json


def _install_patch():
    import concourse.bass as bass

    if getattr(bass.Bass, "_wait_split_installed", False):
        return
    orig = bass.Bass.to_json_bytes

    def to_json_bytes(self):
        return _split_multi_waits_json(orig(self))

    bass.Bass.to_json_bytes = to_json_bytes
    bass.Bass._wait_split_installed = True


# ----------------------------------------------------------------------------
# Problem constants (hardcoded from the problem spec)
# ----------------------------------------------------------------------------
B = 16
N_CORES = 8
B_LOC = B // N_CORES
T_LEN, T_DIM = 149, 768
H = W = 56
S_DIM = 512
N_TOK = H * W           # 3136
CH = 448                # tokens per chunk (8 image rows)
NCHUNK = N_TOK // CH    # 7
NBLK = CH // 112        # 4 two-row attention blocks per chunk
EPS = 1e-5


# ----------------------------------------------------------------------------
# Device program
# ----------------------------------------------------------------------------
def _build_program(apply_g2: bool):
    import concourse.bass as bass
    import concourse.tile as tile
    from concourse import mybir

    F32 = mybir.dt.float32
    BF16 = mybir.dt.bfloat16
    AF = mybir.ActivationFunctionType
    OP = mybir.AluOpType

    nc = bass.Bass(trn_type="TRN2", target_bir_lowering=False, debug=False)
    din = {}
    for name, shape, dt_ in [
        ("x0", (128, B_LOC, T_DIM), BF16), ("x1", (32, B_LOC, T_DIM), BF16),
        ("xdl", (128, 6, B_LOC, T_LEN), BF16),
        ("wqg", (128, 6, S_DIM), BF16),
        ("w1t", (128, 2, N_TOK), BF16),
        ("uq", (1, S_DIM), BF16), ("uk", (1, S_DIM), BF16),
        ("wkt", (128, 4, S_DIM), BF16),
        ("ones", (128, 128), BF16), ("ident", (128, 128), BF16),
        ("cq", (128, 4, N_TOK), BF16), ("ck", (128, 4, N_TOK), BF16),
        ("pe2t", (112, NCHUNK, 4, S_DIM), BF16),
        ("g2bt", (112, S_DIM), BF16),
        ("masks", (112, 112), BF16),
        ("icol", (128, 2), BF16),
        ("ybf", (B_LOC, 128, 4, N_TOK), BF16),
    ]:
        din[name] = nc.dram_tensor(name, list(shape), dt_, kind="ExternalInput").ap()
    dout = nc.dram_tensor("out", [B_LOC, 128, 4, N_TOK], BF16,
                          kind="ExternalOutput").ap()

    from contextlib import ExitStack

    with nc.allow_low_precision(reason="bf16 matmul operands, fp32 accumulate"), \
         tile.TileContext(nc) as tc, ExitStack() as ctx:
        singles = ctx.enter_context(tc.tile_pool(name="singles", bufs=1))
        io3 = ctx.enter_context(tc.tile_pool(name="io3", bufs=3))
        io2 = ctx.enter_context(tc.tile_pool(name="io2", bufs=2))
        wk2 = ctx.enter_context(tc.tile_pool(name="wk2", bufs=2))
        att = ctx.enter_context(tc.tile_pool(name="att", bufs=3))
        sc = ctx.enter_context(tc.tile_pool(name="sc", bufs=2))
        ps_mm = ctx.enter_context(tc.tile_pool(name="ps_mm", bufs=3, space="PSUM"))
        ps_st = ctx.enter_context(tc.tile_pool(name="ps_st", bufs=2, space="PSUM"))
        ps_att = ctx.enter_context(tc.tile_pool(name="ps_att", bufs=2, space="PSUM"))

        def load(name, shape, dt_):
            t = singles.tile(list(shape), dt_, tag=name)
            nc.sync.dma_start(out=t, in_=din[name])
            return t

        x0 = load("x0", (128, B_LOC, T_DIM), BF16)
        x1 = load("x1", (32, B_LOC, T_DIM), BF16)
        xdl = load("xdl", (128, 6, B_LOC, T_LEN), BF16)
        wqg = load("wqg", (128, 6, S_DIM), BF16)
        w1t = load("w1t", (128, 2, N_TOK), BF16)
        uq = load("uq", (1, S_DIM), BF16)
        uk = load("uk", (1, S_DIM), BF16)
        wkt = load("wkt", (128, 4, S_DIM), BF16)
        ones = load("ones", (128, 128), BF16)
        ident = load("ident", (128, 128), BF16)
        g2bt = load("g2bt", (112, S_DIM), BF16)
        masks = load("masks", (112, 112), BF16)
        ones_col = ones[:, 0:1]
        ones_row = ones[0:1, :]
        eps1 = singles.tile([1, 1], F32)
        nc.vector.memset(eps1, EPS)
        onef = singles.tile([1, 1], F32)
        nc.vector.memset(onef, 1.0)
        eps112 = singles.tile([112, 1], F32)
        nc.vector.memset(eps112, EPS)
        icol = load("icol", (128, 2), BF16)

        # ---- per-batch preamble: s_x, Bq = x @ Wqg, G = x x^T ----
        pre = {}
        for b in range(B_LOC):
            sx0 = singles.tile([128, 1], F32, tag=f"sx0_{b}")
            nc.vector.tensor_reduce(out=sx0, in_=x0[:, b, :],
                                    axis=mybir.AxisListType.X, op=OP.add)
            sx1 = singles.tile([32, 1], F32, tag=f"sx1_{b}")
            nc.vector.tensor_reduce(out=sx1, in_=x1[:, b, :],
                                    axis=mybir.AxisListType.X, op=OP.add)
            sxb0 = singles.tile([128, 1], BF16, tag=f"sxb0_{b}")
            nc.scalar.activation(out=sxb0, in_=sx0, func=AF.Copy,
                                 scale=1.0 / T_DIM)
            sxb1 = singles.tile([32, 1], BF16, tag=f"sxb1_{b}")
            nc.scalar.activation(out=sxb1, in_=sx1, func=AF.Copy,
                                 scale=1.0 / T_DIM)

            bq_ps0 = ps_mm.tile([128, S_DIM], F32, tag="mm")
            for dc in range(6):
                nc.tensor.matmul(bq_ps0, xdl[:, dc, b, 0:128], wqg[:, dc, :],
                                 start=(dc == 0), stop=(dc == 5))
            bq0 = singles.tile([128, S_DIM], BF16, tag=f"bq0_{b}")
            nc.scalar.activation(out=bq0, in_=bq_ps0, func=AF.Copy)
            bq_ps1 = ps_mm.tile([21, S_DIM], F32, tag="mm")
            for dc in range(6):
                nc.tensor.matmul(bq_ps1, xdl[:, dc, b, 128:149], wqg[:, dc, :],
                                 start=(dc == 0), stop=(dc == 5))
            bq1 = singles.tile([21, S_DIM], BF16, tag=f"bq1_{b}")
            nc.scalar.activation(out=bq1, in_=bq_ps1, func=AF.Copy)

            g_ps0 = ps_mm.tile([128, T_LEN], F32, tag="mm")
            for dc in range(6):
                nc.tensor.matmul(g_ps0, xdl[:, dc, b, 0:128], xdl[:, dc, b, :],
                                 start=(dc == 0), stop=(dc == 5))
            g0 = singles.tile([128, T_LEN], BF16, tag=f"g0_{b}")
            nc.scalar.activation(out=g0, in_=g_ps0, func=AF.Copy)
            g_ps1 = ps_mm.tile([21, T_LEN], F32, tag="mm")
            for dc in range(6):
                nc.tensor.matmul(g_ps1, xdl[:, dc, b, 128:149], xdl[:, dc, b, :],
                                 start=(dc == 0), stop=(dc == 5))
            g1t = singles.tile([21, T_LEN], BF16, tag=f"g1_{b}")
            nc.scalar.activation(out=g1t, in_=g_ps1, func=AF.Copy)
            pre[b] = (sxb0, sxb1, bq0, bq1, g0, g1t)

        # ---- per chunk x batch phases, software-pipelined over units ----
        def em_ld(u):
            if u["first"]:
                cq_t = io2.tile([128, 4, CH], BF16, tag="cq", bufs=3)
                nc.sync.dma_start(out=cq_t, in_=din["cq"][:, :, u["cols"]])
                ck_t = io2.tile([128, 4, CH], BF16, tag="ck", bufs=3)
                nc.sync.dma_start(out=ck_t, in_=din["ck"][:, :, u["cols"]])
                pe2_t = io2.tile([112, 4, S_DIM], BF16, tag="pe2", bufs=3)
                nc.sync.dma_start(out=pe2_t, in_=din["pe2t"][:, u["ich"], :, :])
                chunk_io[u["ich"]] = (cq_t, ck_t, pe2_t)
            u["cq_t"], u["ck_t"], u["pe2_t"] = chunk_io[u["ich"]]
            ybf = io3.tile([128, 4, CH], BF16, tag="ybf")
            nc.sync.dma_start(out=ybf, in_=din["ybf"][u["b"], :, :, u["cols"]])
            u["ybf"] = ybf

        def em_stats(u):
            b, cols, ybf = u["b"], u["cols"], u["ybf"]
            sxb0, sxb1, bq0, bq1, g0, g1t = pre[b]
            # stat segments in SBUF free dim: 0 = LN1 mean, 1 = LN1 E[x^2]
            st_sb = sc.tile([1, 2, CH], F32, tag="stsb")
            s1p = ps_st.tile([1, CH], F32, tag="st")
            nc.tensor.matmul(s1p, sxb0, w1t[:, 0, cols],
                             start=True, stop=False)
            nc.tensor.matmul(s1p, sxb1[:21, :], w1t[:21, 1, cols],
                             start=False, stop=True)
            nc.scalar.activation(out=st_sb[:, 0, :], in_=s1p, func=AF.Copy)
            # LN1 sumsq via Gram: T = G @ W1T, then colsum(W1T * T)
            t0 = ps_mm.tile([128, CH], F32, tag="mm")
            nc.tensor.matmul(t0, g0[:, 0:128], w1t[:, 0, cols],
                             start=True, stop=False)
            nc.tensor.matmul(t0, g1t[:, 0:128], w1t[:21, 1, cols],
                             start=False, stop=True)
            v0 = wk2.tile([128, CH], BF16, tag="v0")
            nc.vector.tensor_tensor(out=v0, in0=t0, in1=w1t[:, 0, cols],
                                    op=OP.mult)
            t1 = ps_mm.tile([21, CH], F32, tag="mm")
            nc.tensor.matmul(t1, g0[:, 128:149], w1t[:, 0, cols],
                             start=True, stop=False)
            nc.tensor.matmul(t1, g1t[:, 128:149], w1t[:21, 1, cols],
                             start=False, stop=True)
            v1 = wk2.tile([21, CH], BF16, tag="v1")
            nc.vector.tensor_tensor(out=v1, in0=t1, in1=w1t[:21, 1, cols],
                                    op=OP.mult)
            q1p = ps_st.tile([1, CH], F32, tag="st")
            nc.tensor.matmul(q1p, icol[:, 0:1], v0, start=True, stop=False)
            nc.tensor.matmul(q1p, icol[:21, 0:1], v1, start=False, stop=True)
            nc.scalar.activation(out=st_sb[:, 1, :], in_=q1p, func=AF.Copy)
            u["st_sb"] = st_sb

        def em_vt(u):
            ybf = u["ybf"]
            vts = wk2.tile([112, NBLK, S_DIM], BF16, tag="vts")
            m2c = sc.tile([112, 4], F32, tag="m2c")
            q2c = sc.tile([112, 4], F32, tag="q2c")
            for blk in range(NBLK):
                tb = slice(blk * 112, (blk + 1) * 112)
                pt = ps_att.tile([112, S_DIM], BF16, tag="at")
                for co in range(4):
                    nc.tensor.transpose(pt[:, co * 128:(co + 1) * 128],
                                        ybf[:, co, tb], ident)
                nc.scalar.activation(out=vts[:, blk, :], in_=pt, func=AF.Copy,
                                     accum_out=m2c[:, blk:blk + 1])
                scr = att.tile([112, S_DIM], BF16, tag="scr")
                nc.scalar.activation(out=scr, in_=vts[:, blk, :],
                                     func=AF.Square,
                                     accum_out=q2c[:, blk:blk + 1])
            mcn = sc.tile([112, 4], F32, tag="mcn")
            nc.scalar.activation(out=mcn, in_=m2c, func=AF.Copy,
                                 scale=-1.0 / S_DIM)
            msqc = sc.tile([112, 4], F32, tag="msqc")
            nc.vector.tensor_mul(out=msqc, in0=mcn, in1=mcn)
            varc = sc.tile([112, 4], F32, tag="varc")
            nc.vector.tensor_scalar(out=varc, in0=q2c, scalar1=1.0 / S_DIM,
                                    scalar2=None, op0=OP.mult)
            nc.vector.tensor_tensor(out=varc, in0=varc, in1=msqc,
                                    op=OP.subtract)
            lnc = sc.tile([112, 4], F32, tag="lnc")
            nc.scalar.activation(out=lnc, in_=varc, func=AF.Ln, bias=eps112)
            rcol = sc.tile([112, 4], F32, tag="rcol")
            nc.scalar.activation(out=rcol, in_=lnc, func=AF.Exp, scale=-0.5)
            c2col = sc.tile([112, 4], F32, tag="c2col")
            nc.vector.tensor_mul(out=c2col, in0=mcn, in1=rcol)
            rcb = sc.tile([112, 4], BF16, tag="rcb")
            nc.scalar.activation(out=rcb, in_=rcol, func=AF.Copy)
            mcb = sc.tile([112, 4], BF16, tag="mcb")
            nc.scalar.activation(out=mcb, in_=mcn, func=AF.Copy)
            rowp = ps_st.tile([1, 2, CH], BF16, tag="st")
            for blk in range(NBLK):
                tb = slice(blk * 112, (blk + 1) * 112)
                nc.tensor.transpose(rowp[:, 0, tb], rcb[:, blk:blk + 1],
                                    ident[:112, :112])
                nc.tensor.transpose(rowp[:, 1, tb], mcb[:, blk:blk + 1],
                                    ident[:112, :112])
            rowsb = sc.tile([1, 2, CH], BF16, tag="rowsb")
            nc.scalar.activation(out=rowsb, in_=rowp, func=AF.Copy)
            u["vts"], u["rcol"], u["c2col"] = vts, rcol, c2col
            u["rowsb"] = rowsb

        def em_chain(u):
            stp = u["st_sb"]
            mneg = sc.tile([1, 1, CH], BF16, tag="mrow")
            nc.scalar.activation(out=mneg, in_=stp[:, 0:1, :], func=AF.Copy,
                                 scale=-1.0)
            msq = sc.tile([1, 1, CH], F32, tag="msq")
            nc.vector.tensor_mul(out=msq, in0=stp[:, 0:1, :],
                                 in1=stp[:, 0:1, :])
            vrow = sc.tile([1, 1, CH], F32, tag="vrow")
            nc.vector.tensor_tensor(out=vrow, in0=stp[:, 1:2, :], in1=msq,
                                    op=OP.subtract)
            lv = sc.tile([1, 1, CH], F32, tag="lv")
            nc.scalar.activation(out=lv, in_=vrow, func=AF.Ln, bias=eps1)
            rrow = sc.tile([1, 1, CH], BF16, tag="rr")
            nc.scalar.activation(out=rrow, in_=lv, func=AF.Exp, scale=-0.5)
            u["mneg"], u["rrow"] = mneg, rrow

        def em_bcast(u):
            rrow, rowsb = u["rrow"], u["rowsb"]
            rb = wk2.tile([128, 2, CH], BF16, tag="rb")
            r1b_ps = ps_st.tile([128, CH], F32, tag="bc", bufs=1)
            nc.tensor.matmul(r1b_ps, ones_row, rrow[:, 0, :],
                             start=True, stop=True)
            nc.scalar.activation(out=rb[:, 0, :], in_=r1b_ps, func=AF.Copy)
            r2b_ps = ps_st.tile([128, CH], F32, tag="bc", bufs=1)
            nc.tensor.matmul(r2b_ps, ones_row, rowsb[:, 0, :],
                             start=True, stop=True)
            nc.scalar.activation(out=rb[:, 1, :], in_=r2b_ps, func=AF.Copy)
            u["r1b"], u["r2b"] = rb[:, 0, :], rb[:, 1, :]

        def em_q(u):
            b, cols = u["b"], u["cols"]
            mneg, r1b, cq_t = u["mneg"], u["r1b"], u["cq_t"]
            _, _, bq0, bq1, _, _ = pre[b]
            q = wk2.tile([128, 4, CH], BF16, tag="q")
            for oc in range(4):
                ocs = slice(oc * 128, (oc + 1) * 128)
                pq = ps_mm.tile([128, CH], F32, tag="mm")
                nc.tensor.matmul(pq, bq0[:, ocs], w1t[:, 0, cols],
                                 start=True, stop=False)
                nc.tensor.matmul(pq, bq1[:, ocs], w1t[:21, 1, cols],
                                 start=False, stop=False)
                nc.tensor.matmul(pq, uq[:, ocs], mneg[:, 0, :],
                                 start=False, stop=True)
                tmp = att.tile([128, CH], BF16, tag="qt")
                nc.vector.tensor_mul(out=tmp, in0=pq, in1=r1b)
                nc.vector.tensor_add(out=q[:, oc, :], in0=tmp,
                                     in1=cq_t[:, oc, :])
            u["q"] = q

        def em_k(u):
            ybf, r2b, ck_t = u["ybf"], u["r2b"], u["ck_t"]
            m2row = u["rowsb"][:, 1, :]
            k = wk2.tile([128, 4, CH], BF16, tag="k")
            for oc in range(4):
                ocs = slice(oc * 128, (oc + 1) * 128)
                pk = ps_mm.tile([128, CH], F32, tag="mm")
                for kc in range(4):
                    nc.tensor.matmul(pk, wkt[:, kc, ocs], ybf[:, kc, :],
                                     start=(kc == 0), stop=False)
                nc.tensor.matmul(pk, uk[:, ocs], m2row,
                                 start=False, stop=True)
                kt = att.tile([128, CH], BF16, tag="kt")
                nc.vector.tensor_mul(out=kt, in0=pk, in1=r2b)
                nc.vector.tensor_add(out=k[:, oc, :], in0=kt,
                                     in1=ck_t[:, oc, :])
            u["k"] = k

        def em_v(u):
            vts, rcol, c2col = u["vts"], u["rcol"], u["c2col"]
            pe2_t = u["pe2_t"]
            v = wk2.tile([112, NBLK, S_DIM], BF16, tag="v")
            for blk in range(NBLK):
                v1t = att.tile([112, S_DIM], BF16, tag="v1t")
                nc.vector.tensor_scalar(out=v1t, in0=vts[:, blk, :],
                                        scalar1=rcol[:, blk:blk + 1],
                                        scalar2=c2col[:, blk:blk + 1],
                                        op0=OP.mult, op1=OP.add)
                if apply_g2:
                    v1g = att.tile([112, S_DIM], BF16, tag="v1g")
                    nc.vector.tensor_mul(out=v1g, in0=v1t, in1=g2bt)
                    v1t = v1g
                nc.gpsimd.tensor_add(out=v[:, blk, :], in0=v1t,
                                     in1=pe2_t[:, blk, :])
            u["v"] = v

        def em_att_a(u):
            q, k = u["q"], u["k"]
            u["attn"] = []
            for blk in range(NBLK):
                tb = slice(blk * 112, (blk + 1) * 112)
                psc = ps_att.tile([112, 112], F32, tag="at")
                nc.tensor.matmul(psc, ident[:112, :112], masks,
                                 start=True, stop=False)
                for oc in range(4):
                    nc.tensor.matmul(psc, q[:, oc, tb], k[:, oc, tb],
                                     start=False, stop=(oc == 3))
                den = sc.tile([112, 1], F32, tag="den")
                e_b = att.tile([112, 112], BF16, tag="eb")
                nc.scalar.activation(out=e_b, in_=psc, func=AF.Exp,
                                     accum_out=den)
                ld = sc.tile([112, 1], F32, tag="ld")
                nc.scalar.activation(out=ld, in_=den, func=AF.Ln)
                nld = sc.tile([112, 1], F32, tag="nld")
                nc.scalar.activation(out=nld, in_=ld, func=AF.Copy,
                                     scale=-1.0)
                attn = att.tile([112, 112], BF16, tag="attn")
                nc.scalar.activation(out=attn, in_=psc, func=AF.Exp,
                                     bias=nld)
                u["attn"].append(attn)

        def em_att_b(u):
            b, cols, v = u["b"], u["cols"], u["v"]
            out_t = io2.tile([128, 4, CH], BF16, tag="out")
            for blk in range(NBLK):
                tb = slice(blk * 112, (blk + 1) * 112)
                pat = ps_att.tile([112, 112], BF16, tag="at")
                nc.tensor.transpose(pat, u["attn"][blk], ident[:112, :112])
                attnT = att.tile([112, 112], BF16, tag="attnT")
                nc.vector.tensor_copy(out=attnT, in_=pat)
                pav = ps_att.tile([128, 4, 112], F32, tag="at")
                for co in range(4):
                    nc.tensor.matmul(pav[:, co, :],
                                     v[:, blk, co * 128:(co + 1) * 128],
                                     attnT, start=True, stop=True)
                nc.vector.tensor_copy(out=out_t[:, :, tb], in_=pav)
            nc.sync.dma_start(out=dout[b, :, :, cols], in_=out_t)

        units = []
        for ich in range(NCHUNK):
            cols = slice(ich * CH, (ich + 1) * CH)
            for b in range(B_LOC):
                units.append({"b": b, "ich": ich, "cols": cols,
                              "first": b == 0})
        chunk_io = {}
        n = len(units)
        for i in range(-3, n):
            if 0 <= i < n:
                em_att_a(units[i])
                em_att_b(units[i])
            if 0 <= i + 1 < n:
                em_q(units[i + 1])
                em_k(units[i + 1])
                em_v(units[i + 1])
            if 0 <= i + 2 < n:
                em_stats(units[i + 2])
                em_chain(units[i + 2])
                em_bcast(units[i + 2])
            if 0 <= i + 3 < n:
                em_ld(units[i + 3])
                em_vt(units[i + 3])
    return nc


# ----------------------------------------------------------------------------
# Host-side preparation
# ----------------------------------------------------------------------------
def _make_const_inputs(W_conv1, b_conv1, ln1_g, ln1_b, ln2_g, ln2_b,
                       pe_wave, pe_spec, Wq, bq, Wk, bk):
    import ml_dtypes
    f = np.float32
    bf = ml_dtypes.bfloat16
    s = np.float32(S_DIM) ** np.float32(-0.25)

    w1t = np.zeros((128, 2, N_TOK), dtype=f)
    w1T = W_conv1.T.astype(f)
    w1t[:, 0, :] = w1T[:128]
    w1t[:21, 1, :] = w1T[128:]

    # Wqg[d, c] = Wq[c, d] * g1[d] * s, laid out [128, 6, 512]
    wqg = (Wq.T * ln1_g[:, None]).astype(f) * s
    wqg = wqg.reshape(6, 128, S_DIM).transpose(1, 0, 2).copy()
    uq = (Wq @ ln1_g).astype(f)[None, :] * s

    pe_w = pe_wave.reshape(T_DIM, N_TOK).astype(f)
    cq = (Wq @ (ln1_b[:, None] + pe_w)).astype(f) * s + (bq[:, None] * s).astype(f)
    cq = cq.reshape(4, 128, N_TOK).transpose(1, 0, 2).copy()

    # wkt rows scaled by g2 (k-side gamma fold)
    wkt = (Wk.T * (s * ln2_g[:, None])).astype(f)
    wkt = wkt.reshape(4, 128, S_DIM).transpose(1, 0, 2).copy()
    uk = (Wk @ ln2_g).astype(f)[None, :] * s
    apply_g2 = not np.allclose(ln2_g, 1.0)

    pe2_full = (pe_spec.reshape(S_DIM, N_TOK) + ln2_b[:, None]).astype(f)
    ck = ((Wk * ln2_g[None, :]) @ pe2_full) * s + (bk * s)[:, None]
    ck = ck.astype(f).reshape(4, 128, N_TOK).transpose(1, 0, 2).copy()
    pe2t = pe2_full.reshape(S_DIM, NCHUNK, NBLK, 112)
    pe2t = pe2t.transpose(3, 1, 2, 0).copy()
    g2bt = np.broadcast_to(ln2_g[None, :].astype(f), (112, S_DIM)).copy()

    masks = np.full((112, 112), -1e30, dtype=f)
    for sb in range(2):
        masks[sb * 56:(sb + 1) * 56, sb * 56:(sb + 1) * 56] = 0.0

    return {
        "w1t": w1t.astype(bf), "wqg": wqg.astype(bf), "uq": uq.astype(bf),
        "uk": uk.astype(bf), "cq": cq.astype(bf), "wkt": wkt.astype(bf),
        "ck": ck.astype(bf), "pe2t": pe2t.astype(bf),
        "g2bt": g2bt.astype(bf), "masks": masks.astype(bf),
        "ones": np.ones((128, 128), dtype=bf),
        "ident": np.eye(128, dtype=bf),
        "icol": np.stack([np.full(128, 1.0 / T_DIM, dtype=f),
                          np.full(128, 1.0 / S_DIM, dtype=f)],
                         axis=1).astype(bf),
        "_apply_g2": apply_g2,
    }


def _make_core_inputs(consts, x_shard, y_shard):
    import ml_dtypes
    bf = ml_dtypes.bfloat16
    x0 = x_shard[:, :128, :].transpose(1, 0, 2).astype(bf).copy()
    x1 = np.zeros((32, B_LOC, T_DIM), dtype=bf)
    x1[:21] = x_shard[:, 128:, :].transpose(1, 0, 2).astype(bf)
    # x_dl[p, dc, b, l] = x[b, l, dc*128+p]
    xdl = x_shard.transpose(2, 0, 1).reshape(6, 128, B_LOC, T_LEN)
    xdl = xdl.transpose(1, 0, 2, 3).astype(bf).copy()
    ybf = y_shard.reshape(B_LOC, 4, 128, N_TOK).transpose(0, 2, 1, 3)
    ybf = ybf.astype(bf).copy()
    m = {"x0": x0, "x1": x1, "xdl": xdl, "ybf": ybf}
    m.update({k: v for k, v in consts.items() if not k.startswith("_")})
    return m


_cached_nc = [None]


def kernel(x, y, W_conv1, b_conv1, ln1_g, ln1_b, ln2_g, ln2_b,
           pe_wave, pe_spec, Wq, bq, Wk, bk):
    _install_patch()
    from concourse.bass_utils import run_bass_kernel_spmd

    x = np.asarray(x, dtype=np.float32)
    y = np.asarray(y, dtype=np.float32)
    consts = _make_const_inputs(
        np.asarray(W_conv1, np.float32), np.asarray(b_conv1, np.float32),
        np.asarray(ln1_g, np.float32), np.asarray(ln1_b, np.float32),
        np.asarray(ln2_g, np.float32), np.asarray(ln2_b, np.float32),
        np.asarray(pe_wave, np.float32), np.asarray(pe_spec, np.float32),
        np.asarray(Wq, np.float32), np.asarray(bq, np.float32),
        np.asarray(Wk, np.float32), np.asarray(bk, np.float32))
    in_maps = [
        _make_core_inputs(consts, x[B_LOC * i:B_LOC * (i + 1)],
                          y[B_LOC * i:B_LOC * (i + 1)])
        for i in range(N_CORES)
    ]

    if _cached_nc[0] is None:
        _cached_nc[0] = _build_program(consts["_apply_g2"])
    nc = _cached_nc[0]

    res = run_bass_kernel_spmd(nc, in_maps, core_ids=list(range(N_CORES)))
    outs = []
    for i in range(N_CORES):
        o = np.asarray(res.results[i]["out"], dtype=np.float32)
        outs.append(o.transpose(0, 2, 1, 3).reshape(B_LOC, S_DIM, H, W))
    return (np.concatenate(outs, axis=0) + y).astype(np.float32)
